# revision 49
# baseline (speedup 1.0000x reference)
"""Trainium2 Bass kernel for nn_ARP_G_58445914964029.

Computes, per batch b:
    out[b] = sum_{t,j} log p_wrapped_normal(x_err[b,t,j])
for an AR(3) model on the torus (see problem reference). Mathematical
reduction used on device (validated to ~1e-5 rel against the reference):

  dx[t]   = wrap(g[t+1]-g[t])                   (wrap via round-to-nearest)
  wt[t,j] = dx[t+2] - phi0*dx[t+1] - phi1*dx[t] - 2*c_j
  lq      = -0.5*(wt/sig)^2 + ln(1 + e^{|u|-h}) - log_norm,
            u = 2*pi*wt/sig^2,  h = 2*pi^2/sig^2
which equals the reference's 11-term wrapped-normal logsumexp up to
O(e^-79) relative error: wrapping the lead diff g[t+3]-g[t+2] by ITS OWN
nearest multiple of 2*pi (instead of wrapping the full residual) leaves
|wt| <= pi + small, where the 3-shift softplus identity above is valid
(it is valid up to |wt| ~ 8.7), and keeps the Exp/Ln inputs small.

Sharding: data-parallel over the batch axis, one batch per NeuronCore (8).
Host prep per core: g[b] scaled by 1/(2*pi), transposed to [d, t], and laid
out as [128, 2051] f32 (partition p = 32*chunk + dim, 4 time-chunks of 2048,
3-column halo duplicated, tail zero-padded). The 3 phantom tail outputs of
the last chunk are subtracted on the host in f64.

Device per core (all engine work in f32):
  DVE : raws (diff), mu (magic add), nd (wrap residual), B (lag-0 tap),
        wts (lag-1 tap); last subtile's square (as sum(wts^2)+sum(wts))
  ACT : Square+accum, Abs, Exp, Ln+accum  (single natural_log_exp table set)
  DMA : contiguous [128, T] loads (SWDGE), tiny partials store
Host combine in f64 (incl. the bias fixup for the DVE-square subtile and
the 3 phantom tail elements of the last time-chunk).

Measured on 8 axon trn2 cores: HW exec ~30.7-31.2 us warm / ~35.5 us in
the cold-clock band (empty-kernel framework floor is ~15 us; DVE busy
~11 us, ACT busy ~10 us overlapped, DMA ~3 us, rest is entry/exit
barriers, DMA completion latency and pipeline ramp).  Relative error vs
the f32 jax reference: 1.1e-5 (the reference's own f32 rounding noise).

Perf notes: subtile split 128/960/960 with depth-first DVE ordering via
add_dep_helper; single pinned act-table set; loads pre-issued, first on the
scalar HWDGE queue, rest on SWDGE; second exit barrier elided (K_NOBAR2=0 restores it).
"""

import os
import numpy as np
from contextlib import ExitStack

TWO_PI = 2.0 * np.pi
P_AR = 3
N_CORES = 8
MX = 8192
D = 32
CHUNK = 2048          # time steps per partition-chunk
# subtile widths along the 2048-column free axis (must sum to 2048)
SPLIT = [int(x) for x in os.environ.get("K_SPLIT", "128,1088,832").split(",")]
assert sum(SPLIT) == CHUNK
NSUB = len(SPLIT)
HALO = 3
MAGIC = float(np.float32(1.5 * 2**23))  # round-to-nearest magic constant

# R2 (magic add) engine: "act", "dve", or "alt"
R2_ENGINE = os.environ.get("K_R2", "dve")
LAST_RESULTS = None   # test harness introspection

_ACT_SET = "natural_log_exp_and_others"  # contains copy/square/abs/exp/ln


def _pin_act_table_set():
    """Restrict bacc's activation-table choice to one set that covers every
    function this kernel uses, so no ACT_TABLE_LOAD thrashing occurs.  Other
    sets are emptied (not removed) to keep act_func_set_id indices valid."""
    import concourse.hw_specs as hw_specs

    if getattr(hw_specs.get_activation_tables, "_pinned", False):
        return
    orig = hw_specs.get_activation_tables

    def pinned(module_arch):
        tabs = orig(module_arch)
        return {name: (funcs if name == _ACT_SET else set())
                for name, funcs in tabs.items()}

    pinned._pinned = True
    pinned.__wrapped__ = orig
    hw_specs.get_activation_tables = pinned
    # bacc imported the symbol directly; patch there too.
    import concourse.bacc as bacc_mod
    if getattr(bacc_mod, "get_activation_tables", None) is orig:
        bacc_mod.get_activation_tables = pinned


def _install_ntff_hook_shim():
    """Provide antenv.axon_hooks (absent in this image) so that
    run_bass_kernel_spmd(trace=True) can capture NTFF profiles via the
    libaxon ctypes hook from trn_agent_boot. Best-effort."""
    import sys, types
    if "antenv.axon_hooks" in sys.modules:
        return
    try:
        import antenv  # noqa: F401
        mod = types.ModuleType("antenv.axon_hooks")
        mod._hook = None

        def set_axon_ntff_profile_hook(h):
            mod._hook = h

        def get_axon_ntff_profile_hook():
            return mod._hook

        mod.set_axon_ntff_profile_hook = set_axon_ntff_profile_hook
        mod.get_axon_ntff_profile_hook = get_axon_ntff_profile_hook
        sys.modules["antenv.axon_hooks"] = mod
        try:
            from trn_agent_boot.trn_boot import _ntff_profile_via_ctypes
            so = "/opt/axon/libaxon_pjrt.so"
            if os.path.exists(so):
                mod._hook = _ntff_profile_via_ctypes(so)
        except Exception:
            pass
    except Exception:
        pass


def _elide_final_tile_barrier():
    """Drop TileContext's second exit all_engine_barrier: nothing follows it
    in this single-context program and NEFF completion itself waits for all
    engine queues, so it only adds exit latency (~0.4us). Validated correct
    across repeated NEFF executions."""
    import concourse.tile as tile
    from concourse.vector_clock import ScopedClock

    if getattr(tile.TileContext._drain_and_barrier, "_elided", False):
        return

    def _dab(self, tick_clock, wait_clock):
        drain_inst = self.nc.sync.drain()
        wait_clock.add_sem_waits(
            drain_inst.ins, ScopedClock({None: tick_clock.global_clock}))
        self.nc.all_engine_barrier()
        assert self.sems is not None
        popped = self.nc._tile_sem_poison_stack.pop()
        assert popped is self._sem_poison
        self.nc.clear_and_free_semaphores(list(self.sems.allocated().values()))

    _dab._elided = True
    tile.TileContext._drain_and_barrier = _dab


def _device_pass(gs_maps, cvec, phi0, phi1, sigma, trace=False):
    """Build + run the bass program. gs_maps: per-core [128, CHUNK+HALO] f32."""
    import concourse.tile as tile
    from concourse import bacc, mybir
    from concourse.bass_utils import run_bass_kernel_spmd

    if trace:
        _install_ntff_hook_shim()
    _pin_act_table_set()
    if os.environ.get("K_NOBAR2", "1") == "1":
        _elide_final_tile_barrier()

    F = mybir.ActivationFunctionType
    A = mybir.AluOpType
    f32 = mybir.dt.float32

    # wt_full = 2*pi*wts - 2*c_j ; u = 2*pi*wt_full/sig^2
    #   Square((2pi/sig)*wts + (-2c/sig))          = (wt_full/sig)^2
    #   Abs((-(2pi)^2/sig^2)*wts + (4pi/sig^2)*c)  = |-u| = |u|
    # cvec columns: 0: bias_sq, 1: bias_ab, 2: -h, 3: MAGIC
    scale_sq = float(TWO_PI / sigma)
    scale_ab = float(-(TWO_PI ** 2) / sigma ** 2)

    nc = bacc.Bacc("TRN2", target_bir_lowering=False, debug=False,
                   num_devices=N_CORES)
    W = CHUNK + HALO
    gs_in = nc.dram_tensor("gs", [128, W], f32, kind="ExternalInput").ap()
    cv_in = nc.dram_tensor("cvec", [128, 4], f32, kind="ExternalInput").ap()
    # partials columns per subtile i:
    #   col 3i   : sum of Square((2pi/sig)*wts + bias_sq)   (ACT path), or
    #              sum of wts^2                              (DVE path)
    #   col 3i+1 : sum of Ln(1 + e^{|u|-h})
    #   col 3i+2 : sum of wts (DVE path only; for the host-side square fixup)
    part_out = nc.dram_tensor("partials", [128, 3 * NSUB], f32,
                              kind="ExternalOutput").ap()

    with tile.TileContext(nc) as tc, ExitStack() as ctx:
        cpool = ctx.enter_context(tc.tile_pool(name="cpool", bufs=1))
        gpool = ctx.enter_context(tc.tile_pool(name="gpool", bufs=max(2, NSUB)))
        wpool = ctx.enter_context(
            tc.tile_pool(name="wpool", bufs=int(os.environ.get("K_WBUFS", "3"))))
        if os.environ.get("K_PSUM", "0") == "1":
            apool = ctx.enter_context(
                tc.tile_pool(name="apool", bufs=2, space="PSUM"))
        else:
            apool = wpool

        cv = cpool.tile([128, 4], f32, tag="cv")
        nc.sync.dma_start(out=cv[:], in_=cv_in[:])
        pacc = cpool.tile([128, 3 * NSUB], f32, tag="pacc")

        from concourse.tile_rust import add_dep_helper

        # Issue ALL input loads before anything else lands on their engine
        # queues, so the first (latency-critical) load's trigger isn't stuck
        # behind the ACT_TABLE_LOAD on the scalar queue.
        l0 = os.environ.get("K_L0", "scalar")
        gtiles = []
        t_off = 0
        for i, T in enumerate(SPLIT):
            g = gpool.tile([128, T + HALO], f32, tag="g")
            ld = {"sync": nc.sync, "scalar": nc.scalar, "gp": nc.gpsimd}[
                l0 if i == 0 else "gp"]
            ld.dma_start(out=g[:], in_=gs_in[:, t_off: t_off + T + HALO])
            gtiles.append(g)
            t_off += T

        # Dummy early activation: pulls the single ACT_TABLE_LOAD into the
        # first DMA's shadow instead of stalling the first real ACT op.
        warm = cpool.tile([128, 1], f32, tag="warm")
        nc.scalar.activation(out=warm[:], in_=cv[:, 3:4], func=F.Square,
                             bias=0.0, scale=0.0)

        prev_wts_inst = None
        dve_sq_jobs = []
        for i, T in enumerate(SPLIT):
            g = gtiles[i]

            # R1: raws[t] = gs[t+1] - gs[t],  t in [0, T+2)
            raws = wpool.tile([128, T + 2], f32, tag="raws")
            raws_inst = nc.vector.scalar_tensor_tensor(
                out=raws[:], in0=g[:, 0:T + 2], scalar=-1.0, in1=g[:, 1:T + 3],
                op0=A.mult, op1=A.add)
            if prev_wts_inst is not None:
                # Depth-first: keep the DVE finishing subtile i before
                # starting subtile i+1, so ACT starts as early as possible.
                add_dep_helper(raws_inst.ins, prev_wts_inst.ins, sync=False,
                               reason="depth-first subtile order")

            # R2: mu[t] = raws[t] + MAGIC  (rounds: mu = MAGIC + n)
            mu = wpool.tile([128, T + 2], f32, tag="mu")
            if R2_ENGINE == "act" or (R2_ENGINE == "alt" and i % 2 == 0):
                nc.scalar.activation(out=mu[:], in_=raws[:, 0:T + 2],
                                     func=F.Copy, bias=MAGIC, scale=1.0)
            else:
                nc.vector.tensor_scalar(out=mu[:], in0=raws[:, 0:T + 2],
                                        scalar1=MAGIC, scalar2=None, op0=A.add)

            # R3: nd[t] = (mu - MAGIC) - raws = n - raws = -dxs
            nd = wpool.tile([128, T + 2], f32, tag="nd")
            nc.vector.scalar_tensor_tensor(
                out=nd[:], in0=mu[:], scalar=MAGIC, in1=raws[:, 0:T + 2],
                op0=A.subtract, op1=A.subtract)

            # R4: B[t] = phi0*nd[t+1] - nd[t+2]  ( = dxs2 - phi0*dxs1 )
            B = wpool.tile([128, T], f32, tag="B")
            nc.vector.scalar_tensor_tensor(
                out=B[:], in0=nd[:, 1:T + 1], scalar=float(phi0),
                in1=nd[:, 2:T + 2], op0=A.mult, op1=A.subtract)

            dve_sq = (i >= 1)
            # R5: wts[t] = phi1*nd[t] + B  = wt_wrapped/(2*pi)  (no c yet)
            wts = wpool.tile([128, T], f32, tag=f"wts{i}")
            prev_wts_inst = nc.vector.scalar_tensor_tensor(
                out=wts[:], in0=nd[:, 0:T], scalar=float(phi1), in1=B[:],
                op0=A.mult, op1=A.add,
                accum_out=pacc[:, 3 * i + 2:3 * i + 3] if dve_sq else None)

            if dve_sq:
                # Square on DVE, deferred past the final wts (runs concurrent
                # with the ACT tail): sum(wts^2); host reconstructs
                # sum((a*wts+b)^2) from (sum wts^2, sum wts).
                dve_sq_jobs.append((i, wts))
            else:
                # A1: Square((2pi/sig)*wts + bias_sq), accum
                sq = wpool.tile([128, T], f32, tag="sq")
                nc.scalar.activation(out=sq[:], in_=wts[:], func=F.Square,
                                     bias=cv[:, 0:1], scale=scale_sq,
                                     accum_out=pacc[:, 3 * i:3 * i + 1])
            # A2: Abs(scale_ab*wts + bias_ab) = |u|
            ab = apool.tile([128, T], f32, tag="ab")
            nc.scalar.activation(out=ab[:], in_=wts[:], func=F.Abs,
                                 bias=cv[:, 1:2], scale=scale_ab)
            # A3: Exp(|u| - h)
            ex = apool.tile([128, T], f32, tag="ex")
            nc.scalar.activation(out=ex[:], in_=ab[:], func=F.Exp,
                                 bias=cv[:, 2:3], scale=1.0)
            # A4: Ln(1 + ex), accum -> pacc[:, 3i+1]
            ln = wpool.tile([128, T], f32, tag="ln")
            nc.scalar.activation(out=ln[:], in_=ex[:], func=F.Ln,
                                 bias=1.0, scale=1.0,
                                 accum_out=pacc[:, 3 * i + 1:3 * i + 2])
        for i, wtile in reversed(dve_sq_jobs):
            Ti = SPLIT[i]
            sq = wpool.tile([128, Ti], f32, tag="sqd")
            sq_inst = nc.vector.scalar_tensor_tensor(
                out=sq[:], in0=wtile[:], scalar=1.0, in1=wtile[:],
                op0=A.mult, op1=A.mult,
                accum_out=pacc[:, 3 * i:3 * i + 1])
            add_dep_helper(sq_inst.ins, prev_wts_inst.ins, sync=False,
                           reason="defer DVE squares past final wts")

        nc.sync.dma_start(out=part_out[:], in_=pacc[:])

    nc.compile()

    in_maps = [{"gs": gs_maps[c], "cvec": cvec} for c in range(N_CORES)]
    res = run_bass_kernel_spmd(nc, in_maps, list(range(N_CORES)), trace=trace)
    return res


def _reference_fallback(g, ar_c, ar_phi, ar_eta):
    """Exact f64 host fallback (only used if inputs are out of design range)."""
    g = g.astype(np.float64)
    ar_c = ar_c.astype(np.float64)
    phi0, phi1 = float(ar_phi[0, 0]), float(ar_phi[0, 1])
    sigma = abs(float(ar_eta))
    n_b, mx, d = g.shape
    dx = np.mod(g[:, 1:, :] - g[:, :-1, :] + np.pi, TWO_PI) - np.pi
    rp = (g[:, P_AR:, :] - g[:, P_AR - 1:-1, :]
          - phi0 * dx[:, 1:mx - 2, :] - phi1 * dx[:, 0:mx - 3, :]
          - ar_c[None, None, :])
    x_err = np.mod(rp + np.pi, TWO_PI) - np.pi
    v = x_err - ar_c[None, None, :]
    ks = np.arange(-5, 6, dtype=np.float64) * TWO_PI
    z = (v[..., None] + ks) / sigma
    log_norm = np.log(sigma) + 0.5 * np.log(TWO_PI)
    lp = -0.5 * z * z - log_norm
    m = lp.max(axis=-1, keepdims=True)
    lq = m[..., 0] + np.log(np.exp(lp - m).sum(axis=-1))
    return lq.sum(axis=(1, 2)).astype(np.float32)


def kernel(g, ar_c, ar_phi, ar_eta):
    global LAST_RESULTS
    g = np.asarray(g)
    ar_c = np.asarray(ar_c)
    ar_phi = np.asarray(ar_phi).reshape(1, -1)
    ar_eta = np.asarray(ar_eta)

    n_b, mx, d = g.shape
    phi0 = float(ar_phi[0, 0])
    phi1 = float(ar_phi[0, 1])
    sigma = abs(float(ar_eta))
    if sigma == 0.0 or not np.isfinite(sigma):
        return _reference_fallback(g, ar_c, ar_phi, ar_eta)

    # ---- host shard prep ----
    gs = (g.astype(np.float64) / TWO_PI).astype(np.float32)  # scaled
    # range guard: 3-shift truncation needs |wt| <~ 8.7 ; also h must be
    # large enough that the dropped far shift is negligible.
    # Design-range guards (actual data: sigma=0.5, |phi|~1e-3, |c|~4e-3):
    # they keep |wt| <= ~3.3 so the Exp/Ln stages stay in-range, and keep
    # the dropped logsumexp shifts below f32 noise.
    h = 2.0 * np.pi ** 2 / sigma ** 2
    if (n_b != N_CORES or mx != MX or d != D
            or not (0.25 <= sigma <= 0.85)
            or abs(phi0) > 0.01 or abs(phi1) > 0.01
            or np.abs(ar_c).max() > 0.05
            or not np.isfinite(g).all()):
        return _reference_fallback(g, ar_c, ar_phi, ar_eta)

    W = CHUNK + HALO
    gs_maps = []
    for b in range(n_b):
        gt = np.zeros((128, W), dtype=np.float32)
        gsb = gs[b].T  # [32, 8192] (d-major)
        for c in range(4):
            hi = min(MX, c * CHUNK + W)
            gt[c * 32:(c + 1) * 32, : hi - c * CHUNK] = gsb[:, c * CHUNK: hi]
        gs_maps.append(np.ascontiguousarray(gt))

    # ---- per-partition constants (p = 32*chunk + j) ----
    cj = np.tile(ar_c.astype(np.float64), 4)  # [128]
    # device x_dev = wts (NOT sign flipped in this implementation):
    #   wt_full = 2*pi*wts - 2*c_j
    # Square input: (2pi/sig)*wts + bias_sq  -> (wt_full/sig)  => bias_sq = -2c/sig
    # Abs input:   scale_ab*wts + bias_ab   -> -(u)            => |.| = |u|
    #   u = (2pi/sig^2)*wt_full = (4pi^2/sig^2)*wts - (4pi/sig^2)*c
    #   scale_ab = -(4pi^2/sig^2)  => bias_ab = +(4pi/sig^2)*c  (negated u; abs ok)
    cvec = np.zeros((128, 4), dtype=np.float32)
    cvec[:, 0] = (-2.0 * cj / sigma).astype(np.float32)
    cvec[:, 1] = (2.0 * TWO_PI / sigma ** 2 * cj).astype(np.float32)
    cvec[:, 2] = np.float32(-h)
    cvec[:, 3] = np.float32(MAGIC)

    trace = bool(os.environ.get("BASS_TRACE"))
    res = _device_pass(gs_maps, cvec, phi0, phi1, sigma, trace=trace)
    LAST_RESULTS = res

    # ---- host combine (f64) ----
    log_norm = np.log(sigma) + 0.5 * np.log(TWO_PI)
    n_valid = (MX - P_AR) * D
    a_sq = TWO_PI / sigma                 # Square input scale
    b_sq = -2.0 * cj / sigma              # per-partition Square bias [128]
    T_last = SPLIT[-1]
    out = np.zeros(n_b, dtype=np.float64)
    for b in range(n_b):
        pa = res.results[b]["partials"].astype(np.float64)  # [128, 3*NSUB]
        sq_sum = pa[:, 0].sum()                    # subtile 0 (ACT path)
        ln_sum = pa[:, 1::3].sum()

        # phantom tail outputs of chunk 3 (partitions 96..127) are the final
        # 3 columns of the LAST subtile; remove their contributions.
        gt = gs_maps[b].astype(np.float64)
        tail = gt[96:, -9:]  # [32, 9] columns 2042..2050
        raws = tail[:, 1:] - tail[:, :-1]            # [32, 8]
        nd = np.round(raws) - raws                   # = -dxs
        wts_all = -nd[:, 2:] + phi0 * nd[:, 1:-1] + phi1 * nd[:, :-2]  # [32,6]
        wts_ph = wts_all[:, -3:]                     # t_local 2045..2047

        for i in range(1, NSUB):                   # DVE-square subtiles
            s2 = pa[:, 3 * i + 0].copy()           # sum wts^2
            s1 = pa[:, 3 * i + 2].copy()           # sum wts
            n_j = np.full(128, float(SPLIT[i]))
            if i == NSUB - 1:
                s2[96:] -= (wts_ph ** 2).sum(axis=1)
                s1[96:] -= wts_ph.sum(axis=1)
                n_j[96:] -= 3.0
            sq_sum += (a_sq ** 2 * s2 + 2.0 * a_sq * b_sq * s1
                       + n_j * b_sq ** 2).sum()

        wt_ph = TWO_PI * wts_ph - 2.0 * cj[96:, None]
        ln_corr = np.log1p(np.exp(np.abs(TWO_PI * wt_ph / sigma ** 2) - h))
        ln_sum -= ln_corr.sum()

        out[b] = -0.5 * sq_sum + ln_sum - n_valid * log_norm
    return out.astype(np.float32)


# revision 52
# speedup vs baseline: 1.1518x; 1.1518x over previous
"""Trainium2 Bass kernel for nn_ARP_G_58445914964029.

Computes, per batch b:
    out[b] = sum_{t,j} log p_wrapped_normal(x_err[b,t,j])
for an AR(3) model on the torus (see problem reference). Mathematical
reduction used on device (validated to ~1e-5 rel against the reference):

  dx[t]   = wrap(g[t+1]-g[t])                   (wrap via round-to-nearest)
  wt[t,j] = dx[t+2] - phi0*dx[t+1] - phi1*dx[t] - 2*c_j
  lq      = -0.5*(wt/sig)^2 + ln(1 + e^{|u|-h}) - log_norm,
            u = 2*pi*wt/sig^2,  h = 2*pi^2/sig^2
which equals the reference's 11-term wrapped-normal logsumexp up to
O(e^-79) relative error: wrapping the lead diff g[t+3]-g[t+2] by ITS OWN
nearest multiple of 2*pi (instead of wrapping the full residual) leaves
|wt| <= pi + small, where the 3-shift softplus identity above is valid
(it is valid up to |wt| ~ 8.7), and keeps the Exp/Ln inputs small.

Sharding: data-parallel over the batch axis, one batch per NeuronCore (8).
Host prep per core: g[b] scaled by 1/(2*pi), transposed to [d, t], and laid
out as [128, 2051] f32 (partition p = 32*chunk + dim, 4 time-chunks of 2048,
3-column halo duplicated, tail zero-padded). The 3 phantom tail outputs of
the last chunk are subtracted on the host in f64.

Device per core (all engine work in f32):
  DVE : raws (diff), mu (magic add), nd (wrap residual), B (lag-0 tap),
        wts (lag-1 tap); ALL squares deferred past the final wts (as
        sum(wts^2)+sum(wts) with host fixup, concurrent with the ACT tail)
  ACT : Abs, Exp, Ln+accum  (single natural_log_exp table set)
  DMA : contiguous [128, T] loads (SWDGE), tiny partials store
Host combine in f64 (incl. the bias fixup for the DVE-square subtile and
the 3 phantom tail elements of the last time-chunk).

Measured on 8 axon trn2 cores: HW exec ~30.7-31.2 us warm / ~35.5 us in
the cold-clock band (empty-kernel framework floor is ~15 us; DVE busy
~11 us, ACT busy ~10 us overlapped, DMA ~3 us, rest is entry/exit
barriers, DMA completion latency and pipeline ramp).  Relative error vs
the f32 jax reference: 1.1e-5 (the reference's own f32 rounding noise).

Perf notes: subtile split 128/1152/768, depth-first DVE ordering via
add_dep_helper; single pinned act-table set; loads pre-issued, first on the
scalar HWDGE queue, rest on SWDGE; second exit barrier elided (K_NOBAR2=0 restores it).
"""

import os
import numpy as np
from contextlib import ExitStack

TWO_PI = 2.0 * np.pi
P_AR = 3
N_CORES = 8
MX = 8192
D = 32
CHUNK = 2048          # time steps per partition-chunk
# subtile widths along the 2048-column free axis (must sum to 2048)
SPLIT = [int(x) for x in os.environ.get("K_SPLIT", "128,1152,768").split(",")]
assert sum(SPLIT) == CHUNK
NSUB = len(SPLIT)
HALO = 3
MAGIC = float(np.float32(1.5 * 2**23))  # round-to-nearest magic constant

# R2 (magic add) engine: "act", "dve", or "alt"
R2_ENGINE = os.environ.get("K_R2", "dve")
LAST_RESULTS = None   # test harness introspection

_ACT_SET = "natural_log_exp_and_others"  # contains copy/square/abs/exp/ln


def _pin_act_table_set():
    """Restrict bacc's activation-table choice to one set that covers every
    function this kernel uses, so no ACT_TABLE_LOAD thrashing occurs.  Other
    sets are emptied (not removed) to keep act_func_set_id indices valid."""
    import concourse.hw_specs as hw_specs

    if getattr(hw_specs.get_activation_tables, "_pinned", False):
        return
    orig = hw_specs.get_activation_tables

    def pinned(module_arch):
        tabs = orig(module_arch)
        return {name: (funcs if name == _ACT_SET else set())
                for name, funcs in tabs.items()}

    pinned._pinned = True
    pinned.__wrapped__ = orig
    hw_specs.get_activation_tables = pinned
    # bacc imported the symbol directly; patch there too.
    import concourse.bacc as bacc_mod
    if getattr(bacc_mod, "get_activation_tables", None) is orig:
        bacc_mod.get_activation_tables = pinned


def _install_ntff_hook_shim():
    """Provide antenv.axon_hooks (absent in this image) so that
    run_bass_kernel_spmd(trace=True) can capture NTFF profiles via the
    libaxon ctypes hook from trn_agent_boot. Best-effort."""
    import sys, types
    if "antenv.axon_hooks" in sys.modules:
        return
    try:
        import antenv  # noqa: F401
        mod = types.ModuleType("antenv.axon_hooks")
        mod._hook = None

        def set_axon_ntff_profile_hook(h):
            mod._hook = h

        def get_axon_ntff_profile_hook():
            return mod._hook

        mod.set_axon_ntff_profile_hook = set_axon_ntff_profile_hook
        mod.get_axon_ntff_profile_hook = get_axon_ntff_profile_hook
        sys.modules["antenv.axon_hooks"] = mod
        try:
            from trn_agent_boot.trn_boot import _ntff_profile_via_ctypes
            so = "/opt/axon/libaxon_pjrt.so"
            if os.path.exists(so):
                mod._hook = _ntff_profile_via_ctypes(so)
        except Exception:
            pass
    except Exception:
        pass


def _elide_final_tile_barrier():
    """Drop TileContext's second exit all_engine_barrier: nothing follows it
    in this single-context program and NEFF completion itself waits for all
    engine queues, so it only adds exit latency (~0.4us). Validated correct
    across repeated NEFF executions."""
    import concourse.tile as tile
    from concourse.vector_clock import ScopedClock

    if getattr(tile.TileContext._drain_and_barrier, "_elided", False):
        return

    def _dab(self, tick_clock, wait_clock):
        drain_inst = self.nc.sync.drain()
        wait_clock.add_sem_waits(
            drain_inst.ins, ScopedClock({None: tick_clock.global_clock}))
        self.nc.all_engine_barrier()
        assert self.sems is not None
        popped = self.nc._tile_sem_poison_stack.pop()
        assert popped is self._sem_poison
        self.nc.clear_and_free_semaphores(list(self.sems.allocated().values()))

    _dab._elided = True
    tile.TileContext._drain_and_barrier = _dab


def _device_pass(gs_maps, cvec, phi0, phi1, sigma, trace=False):
    """Build + run the bass program. gs_maps: per-core [128, CHUNK+HALO] f32."""
    import concourse.tile as tile
    from concourse import bacc, mybir
    from concourse.bass_utils import run_bass_kernel_spmd

    if trace:
        _install_ntff_hook_shim()
    _pin_act_table_set()
    if os.environ.get("K_NOBAR2", "1") == "1":
        _elide_final_tile_barrier()

    F = mybir.ActivationFunctionType
    A = mybir.AluOpType
    f32 = mybir.dt.float32

    # wt_full = 2*pi*wts - 2*c_j ; u = 2*pi*wt_full/sig^2
    #   Square((2pi/sig)*wts + (-2c/sig))          = (wt_full/sig)^2
    #   Abs((-(2pi)^2/sig^2)*wts + (4pi/sig^2)*c)  = |-u| = |u|
    # cvec columns: 0: bias_sq, 1: bias_ab, 2: -h, 3: MAGIC
    scale_sq = float(TWO_PI / sigma)
    scale_ab = float(-(TWO_PI ** 2) / sigma ** 2)

    nc = bacc.Bacc("TRN2", target_bir_lowering=False, debug=False,
                   num_devices=N_CORES)
    W = CHUNK + HALO
    gs_in = nc.dram_tensor("gs", [128, W], f32, kind="ExternalInput").ap()
    cv_in = nc.dram_tensor("cvec", [128, 4], f32, kind="ExternalInput").ap()
    # partials columns per subtile i:
    #   col 3i   : sum of Square((2pi/sig)*wts + bias_sq)   (ACT path), or
    #              sum of wts^2                              (DVE path)
    #   col 3i+1 : sum of Ln(1 + e^{|u|-h})
    #   col 3i+2 : sum of wts (DVE path only; for the host-side square fixup)
    part_out = nc.dram_tensor("partials", [128, 3 * NSUB], f32,
                              kind="ExternalOutput").ap()

    with tile.TileContext(nc) as tc, ExitStack() as ctx:
        cpool = ctx.enter_context(tc.tile_pool(name="cpool", bufs=1))
        gpool = ctx.enter_context(tc.tile_pool(name="gpool", bufs=max(2, NSUB)))
        wpool = ctx.enter_context(
            tc.tile_pool(name="wpool", bufs=int(os.environ.get("K_WBUFS", "3"))))
        if os.environ.get("K_PSUM", "0") == "1":
            apool = ctx.enter_context(
                tc.tile_pool(name="apool", bufs=2, space="PSUM"))
        else:
            apool = wpool

        cv = cpool.tile([128, 4], f32, tag="cv")
        nc.sync.dma_start(out=cv[:], in_=cv_in[:])
        pacc = cpool.tile([128, 3 * NSUB], f32, tag="pacc")

        from concourse.tile_rust import add_dep_helper

        # Issue ALL input loads before anything else lands on their engine
        # queues, so the first (latency-critical) load's trigger isn't stuck
        # behind the ACT_TABLE_LOAD on the scalar queue.
        l0 = os.environ.get("K_L0", "scalar")
        gtiles = []
        t_off = 0
        for i, T in enumerate(SPLIT):
            g = gpool.tile([128, T + HALO], f32, tag="g")
            ld = {"sync": nc.sync, "scalar": nc.scalar, "gp": nc.gpsimd}[
                l0 if i == 0 else "gp"]
            ld.dma_start(out=g[:], in_=gs_in[:, t_off: t_off + T + HALO])
            gtiles.append(g)
            t_off += T

        # Dummy early activation: pulls the single ACT_TABLE_LOAD into the
        # first DMA's shadow instead of stalling the first real ACT op.
        warm = cpool.tile([128, 1], f32, tag="warm")
        nc.scalar.activation(out=warm[:], in_=cv[:, 3:4], func=F.Square,
                             bias=0.0, scale=0.0)

        prev_wts_inst = None
        dve_sq_jobs = []
        for i, T in enumerate(SPLIT):
            g = gtiles[i]

            # R1: raws[t] = gs[t+1] - gs[t],  t in [0, T+2)
            raws = wpool.tile([128, T + 2], f32, tag="raws")
            raws_inst = nc.vector.scalar_tensor_tensor(
                out=raws[:], in0=g[:, 0:T + 2], scalar=-1.0, in1=g[:, 1:T + 3],
                op0=A.mult, op1=A.add)
            if prev_wts_inst is not None:
                # Depth-first: keep the DVE finishing subtile i before
                # starting subtile i+1, so ACT starts as early as possible.
                add_dep_helper(raws_inst.ins, prev_wts_inst.ins, sync=False,
                               reason="depth-first subtile order")

            # R2: mu[t] = raws[t] + MAGIC  (rounds: mu = MAGIC + n)
            mu = wpool.tile([128, T + 2], f32, tag="mu")
            if R2_ENGINE == "act" or (R2_ENGINE == "alt" and i % 2 == 0):
                nc.scalar.activation(out=mu[:], in_=raws[:, 0:T + 2],
                                     func=F.Copy, bias=MAGIC, scale=1.0)
            else:
                nc.vector.tensor_scalar(out=mu[:], in0=raws[:, 0:T + 2],
                                        scalar1=MAGIC, scalar2=None, op0=A.add)

            # R3: nd[t] = (mu - MAGIC) - raws = n - raws = -dxs
            nd = wpool.tile([128, T + 2], f32, tag="nd")
            nc.vector.scalar_tensor_tensor(
                out=nd[:], in0=mu[:], scalar=MAGIC, in1=raws[:, 0:T + 2],
                op0=A.subtract, op1=A.subtract)

            # R4: B[t] = phi0*nd[t+1] - nd[t+2]  ( = dxs2 - phi0*dxs1 )
            B = wpool.tile([128, T], f32, tag="B")
            nc.vector.scalar_tensor_tensor(
                out=B[:], in0=nd[:, 1:T + 1], scalar=float(phi0),
                in1=nd[:, 2:T + 2], op0=A.mult, op1=A.subtract)

            dve_sq = (i >= 1) or os.environ.get("K_SQ0", "dve") == "dve"
            # R5: wts[t] = phi1*nd[t] + B  = wt_wrapped/(2*pi)  (no c yet)
            wts = wpool.tile([128, T], f32, tag=f"wts{i}")
            prev_wts_inst = nc.vector.scalar_tensor_tensor(
                out=wts[:], in0=nd[:, 0:T], scalar=float(phi1), in1=B[:],
                op0=A.mult, op1=A.add,
                accum_out=pacc[:, 3 * i + 2:3 * i + 3] if dve_sq else None)

            if dve_sq:
                # Square on DVE, deferred past the final wts (runs concurrent
                # with the ACT tail): sum(wts^2); host reconstructs
                # sum((a*wts+b)^2) from (sum wts^2, sum wts).
                dve_sq_jobs.append((i, wts))
            else:
                # A1: Square((2pi/sig)*wts + bias_sq), accum
                sq = wpool.tile([128, T], f32, tag="sq")
                nc.scalar.activation(out=sq[:], in_=wts[:], func=F.Square,
                                     bias=cv[:, 0:1], scale=scale_sq,
                                     accum_out=pacc[:, 3 * i:3 * i + 1])
            # A2: Abs(scale_ab*wts + bias_ab) = |u|
            ab = apool.tile([128, T], f32, tag="ab")
            nc.scalar.activation(out=ab[:], in_=wts[:], func=F.Abs,
                                 bias=cv[:, 1:2], scale=scale_ab)
            # A3: Exp(|u| - h)
            ex = apool.tile([128, T], f32, tag="ex")
            nc.scalar.activation(out=ex[:], in_=ab[:], func=F.Exp,
                                 bias=cv[:, 2:3], scale=1.0)
            # A4: Ln(1 + ex), accum -> pacc[:, 3i+1]
            ln = wpool.tile([128, T], f32, tag="ln")
            nc.scalar.activation(out=ln[:], in_=ex[:], func=F.Ln,
                                 bias=1.0, scale=1.0,
                                 accum_out=pacc[:, 3 * i + 1:3 * i + 2])
        for i, wtile in reversed(dve_sq_jobs):
            Ti = SPLIT[i]
            sq = wpool.tile([128, Ti], f32, tag="sqd")
            sq_inst = nc.vector.scalar_tensor_tensor(
                out=sq[:], in0=wtile[:], scalar=1.0, in1=wtile[:],
                op0=A.mult, op1=A.mult,
                accum_out=pacc[:, 3 * i:3 * i + 1])
            add_dep_helper(sq_inst.ins, prev_wts_inst.ins, sync=False,
                           reason="defer DVE squares past final wts")

        nc.sync.dma_start(out=part_out[:], in_=pacc[:])

    nc.compile()

    in_maps = [{"gs": gs_maps[c], "cvec": cvec} for c in range(N_CORES)]
    res = run_bass_kernel_spmd(nc, in_maps, list(range(N_CORES)), trace=trace)
    return res


def _reference_fallback(g, ar_c, ar_phi, ar_eta):
    """Exact f64 host fallback (only used if inputs are out of design range)."""
    g = g.astype(np.float64)
    ar_c = ar_c.astype(np.float64)
    phi0, phi1 = float(ar_phi[0, 0]), float(ar_phi[0, 1])
    sigma = abs(float(ar_eta))
    n_b, mx, d = g.shape
    dx = np.mod(g[:, 1:, :] - g[:, :-1, :] + np.pi, TWO_PI) - np.pi
    rp = (g[:, P_AR:, :] - g[:, P_AR - 1:-1, :]
          - phi0 * dx[:, 1:mx - 2, :] - phi1 * dx[:, 0:mx - 3, :]
          - ar_c[None, None, :])
    x_err = np.mod(rp + np.pi, TWO_PI) - np.pi
    v = x_err - ar_c[None, None, :]
    ks = np.arange(-5, 6, dtype=np.float64) * TWO_PI
    z = (v[..., None] + ks) / sigma
    log_norm = np.log(sigma) + 0.5 * np.log(TWO_PI)
    lp = -0.5 * z * z - log_norm
    m = lp.max(axis=-1, keepdims=True)
    lq = m[..., 0] + np.log(np.exp(lp - m).sum(axis=-1))
    return lq.sum(axis=(1, 2)).astype(np.float32)


def kernel(g, ar_c, ar_phi, ar_eta):
    global LAST_RESULTS
    g = np.asarray(g)
    ar_c = np.asarray(ar_c)
    ar_phi = np.asarray(ar_phi).reshape(1, -1)
    ar_eta = np.asarray(ar_eta)

    n_b, mx, d = g.shape
    phi0 = float(ar_phi[0, 0])
    phi1 = float(ar_phi[0, 1])
    sigma = abs(float(ar_eta))
    if sigma == 0.0 or not np.isfinite(sigma):
        return _reference_fallback(g, ar_c, ar_phi, ar_eta)

    # ---- host shard prep ----
    gs = (g.astype(np.float64) / TWO_PI).astype(np.float32)  # scaled
    # range guard: 3-shift truncation needs |wt| <~ 8.7 ; also h must be
    # large enough that the dropped far shift is negligible.
    # Design-range guards (actual data: sigma=0.5, |phi|~1e-3, |c|~4e-3):
    # they keep |wt| <= ~3.3 so the Exp/Ln stages stay in-range, and keep
    # the dropped logsumexp shifts below f32 noise.
    h = 2.0 * np.pi ** 2 / sigma ** 2
    if (n_b != N_CORES or mx != MX or d != D
            or not (0.25 <= sigma <= 0.85)
            or abs(phi0) > 0.01 or abs(phi1) > 0.01
            or np.abs(ar_c).max() > 0.05
            or not np.isfinite(g).all()):
        return _reference_fallback(g, ar_c, ar_phi, ar_eta)

    W = CHUNK + HALO
    gs_maps = []
    for b in range(n_b):
        gt = np.zeros((128, W), dtype=np.float32)
        gsb = gs[b].T  # [32, 8192] (d-major)
        for c in range(4):
            hi = min(MX, c * CHUNK + W)
            gt[c * 32:(c + 1) * 32, : hi - c * CHUNK] = gsb[:, c * CHUNK: hi]
        gs_maps.append(np.ascontiguousarray(gt))

    # ---- per-partition constants (p = 32*chunk + j) ----
    cj = np.tile(ar_c.astype(np.float64), 4)  # [128]
    # device x_dev = wts (NOT sign flipped in this implementation):
    #   wt_full = 2*pi*wts - 2*c_j
    # Square input: (2pi/sig)*wts + bias_sq  -> (wt_full/sig)  => bias_sq = -2c/sig
    # Abs input:   scale_ab*wts + bias_ab   -> -(u)            => |.| = |u|
    #   u = (2pi/sig^2)*wt_full = (4pi^2/sig^2)*wts - (4pi/sig^2)*c
    #   scale_ab = -(4pi^2/sig^2)  => bias_ab = +(4pi/sig^2)*c  (negated u; abs ok)
    cvec = np.zeros((128, 4), dtype=np.float32)
    cvec[:, 0] = (-2.0 * cj / sigma).astype(np.float32)
    cvec[:, 1] = (2.0 * TWO_PI / sigma ** 2 * cj).astype(np.float32)
    cvec[:, 2] = np.float32(-h)
    cvec[:, 3] = np.float32(MAGIC)

    trace = bool(os.environ.get("BASS_TRACE"))
    res = _device_pass(gs_maps, cvec, phi0, phi1, sigma, trace=trace)
    LAST_RESULTS = res

    # ---- host combine (f64) ----
    log_norm = np.log(sigma) + 0.5 * np.log(TWO_PI)
    n_valid = (MX - P_AR) * D
    a_sq = TWO_PI / sigma                 # Square input scale
    b_sq = -2.0 * cj / sigma              # per-partition Square bias [128]
    T_last = SPLIT[-1]
    out = np.zeros(n_b, dtype=np.float64)
    for b in range(n_b):
        pa = res.results[b]["partials"].astype(np.float64)  # [128, 3*NSUB]
        sq_sum = pa[:, 0].sum()                    # subtile 0 (ACT path)
        ln_sum = pa[:, 1::3].sum()

        # phantom tail outputs of chunk 3 (partitions 96..127) are the final
        # 3 columns of the LAST subtile; remove their contributions.
        gt = gs_maps[b].astype(np.float64)
        tail = gt[96:, -9:]  # [32, 9] columns 2042..2050
        raws = tail[:, 1:] - tail[:, :-1]            # [32, 8]
        nd = np.round(raws) - raws                   # = -dxs
        wts_all = -nd[:, 2:] + phi0 * nd[:, 1:-1] + phi1 * nd[:, :-2]  # [32,6]
        wts_ph = wts_all[:, -3:]                     # t_local 2045..2047

        i0_dve = os.environ.get("K_SQ0", "dve") == "dve"
        sq_sum = 0.0 if i0_dve else sq_sum
        for i in range(0 if i0_dve else 1, NSUB):  # DVE-square subtiles
            s2 = pa[:, 3 * i + 0].copy()           # sum wts^2
            s1 = pa[:, 3 * i + 2].copy()           # sum wts
            n_j = np.full(128, float(SPLIT[i]))
            if i == NSUB - 1:
                s2[96:] -= (wts_ph ** 2).sum(axis=1)
                s1[96:] -= wts_ph.sum(axis=1)
                n_j[96:] -= 3.0
            sq_sum += (a_sq ** 2 * s2 + 2.0 * a_sq * b_sq * s1
                       + n_j * b_sq ** 2).sum()

        wt_ph = TWO_PI * wts_ph - 2.0 * cj[96:, None]
        ln_corr = np.log1p(np.exp(np.abs(TWO_PI * wt_ph / sigma ** 2) - h))
        ln_sum -= ln_corr.sum()

        out[b] = -0.5 * sq_sum + ln_sum - n_valid * log_norm
    return out.astype(np.float32)
